# revision 5
# baseline (speedup 1.0000x reference)
"""Multi-head attention (B=4, S=2048, D=768, H=12) on 8 Trainium2 NeuronCores.

Sharding: batch x head-group. Core c handles batch b = c//2 and head group
g = c%2 (6 heads of 64 dims each). Each core computes its heads' QKV
projections, attention, and a partial output projection (contracting only its
384 head dims of w_proj). The host sums the two partial projections per batch
and adds b_proj.

Per-core device pipeline (all matmuls in fp32r = full-rate FP22):
  1. QKV: q^T,k^T stored [head_dim, S] (head dim on partitions), v stored
     [S, head_dim] with a ones column appended per head (for softmax sums).
  2. Attention per head pair (row-tiled K=64 matmuls share the PE array):
     S^T tile = k^T.T @ q^T -> PSUM, exp via ScalarE (scale=1/8 folded in),
     then out^T[65, q] += [v|1].T @ P^T accumulated over kv tiles; row 64 of
     out^T is the softmax denominator. Normalize with reciprocal +
     partition-broadcast + multiply into oh^T [head_dim, S].
  3. proj partial: y = oh^T.T @ w_proj_rows, DVE copy PSUM->SBUF, DMA out.
"""

import numpy as np

B, S, D = 4, 2048, 768
H, HD = 12, 64
HPG = 6          # heads per group (per core)
NCORES = 8
KT = S // 128    # 16 kv tiles
QC = 4           # q chunks of 512
VW = HPG * (HD + 1)   # 390: v with ones column per head

_compiled = {}
LAST_RESULT = None


def _build_nc():
    import concourse.bass as bass
    import concourse.mybir as mybir
    import concourse.tile as tile
    from concourse import bacc

    f32 = mybir.dt.float32
    f32r = mybir.dt.float32r
    Exp = mybir.ActivationFunctionType.Exp

    nc = bacc.Bacc("TRN2", target_bir_lowering=False, debug=False,
                   num_devices=NCORES)

    xT = nc.dram_tensor("xT", [D, S], f32r, kind="ExternalInput")
    wq = nc.dram_tensor("wq", [D, 384], f32r, kind="ExternalInput")
    wk = nc.dram_tensor("wk", [D, 384], f32r, kind="ExternalInput")
    wv = nc.dram_tensor("wv", [D, VW], f32r, kind="ExternalInput")
    bq = nc.dram_tensor("bq", [384], f32, kind="ExternalInput")
    bk = nc.dram_tensor("bk", [384], f32, kind="ExternalInput")
    bv = nc.dram_tensor("bv", [VW], f32r, kind="ExternalInput")
    wp = nc.dram_tensor("wp", [384, D], f32r, kind="ExternalInput")
    y = nc.dram_tensor("y", [S, D], f32, kind="ExternalOutput")

    with tile.TileContext(nc) as tc:
        with tc.tile_pool(name="singles", bufs=1) as singles, \
             tc.tile_pool(name="pt", bufs=2) as ptp, \
             tc.tile_pool(name="norm", bufs=2) as normp, \
             tc.tile_pool(name="yout", bufs=2) as youtp, \
             tc.tile_pool(name="big", bufs=2, space="PSUM") as bigps, \
             tc.tile_pool(name="ov", bufs=2, space="PSUM") as ovps:

            # ---- static SBUF tensors ----
            xT_sb = singles.tile([128, 6, S], f32r)
            wq_sb = singles.tile([128, 6, 384], f32r)
            wk_sb = singles.tile([128, 6, 384], f32r)
            wv_sb = singles.tile([128, 6, VW], f32r)
            wp_sb = singles.tile([128, 3, D], f32r)
            bq_sb = singles.tile([128, 3], f32)
            bk_sb = singles.tile([128, 3], f32)
            bv_sb = singles.tile([1, VW], f32r)
            ones_col = singles.tile([1, 128], f32r)
            qT_sb = singles.tile([128, 3, S], f32r)
            kT_sb = singles.tile([128, 3, S], f32r)
            v_sb = singles.tile([128, KT, VW], f32r)
            ohT_sb = singles.tile([128, 3, S], f32r)

            xT_r = xT.ap().rearrange("(t p) q -> p t q", p=128)
            wq_r = wq.ap().rearrange("(t p) m -> p t m", p=128)
            wk_r = wk.ap().rearrange("(t p) m -> p t m", p=128)
            wv_r = wv.ap().rearrange("(t p) m -> p t m", p=128)
            wp_r = wp.ap().rearrange("(t p) m -> p t m", p=128)
            for t in range(6):
                nc.sync.dma_start(xT_sb[:, t, :], xT_r[:, t, :])
                nc.sync.dma_start(wq_sb[:, t, :], wq_r[:, t, :])
                nc.sync.dma_start(wk_sb[:, t, :], wk_r[:, t, :])
                nc.sync.dma_start(wv_sb[:, t, :], wv_r[:, t, :])
            for t in range(3):
                nc.sync.dma_start(wp_sb[:, t, :], wp_r[:, t, :])
            nc.sync.dma_start(bq_sb[:], bq.ap().rearrange("(t p) -> p t", p=128))
            nc.sync.dma_start(bk_sb[:], bk.ap().rearrange("(t p) -> p t", p=128))
            nc.sync.dma_start(bv_sb[:], bv.ap()[None, :])
            ones_f32 = singles.tile([1, 128], f32)
            nc.vector.memset(ones_f32[:], 1.0)
            nc.vector.tensor_copy(out=ones_col[:], in_=ones_f32[:])

            # ---- v = x @ wv + bv (ones columns arrive via the bv row) ----
            for s in range(KT):
                vps = bigps.tile([128, 1024], f32, tag="big")
                nc.tensor.matmul(vps[:, :VW], ones_col[:], bv_sb[:],
                                 start=True, stop=False)
                for k in range(6):
                    nc.tensor.matmul(vps[:, :VW],
                                     xT_sb[:, k, s * 128:(s + 1) * 128],
                                     wv_sb[:, k, :],
                                     start=False, stop=(k == 5))
                nc.vector.tensor_copy(out=v_sb[:, s, :], in_=vps[:, :VW])

            for hp in range(3):
                # ---- q^T, k^T for this head pair ----
                for (w_sb, b_sb, dst) in ((wq_sb, bq_sb, qT_sb),
                                          (wk_sb, bk_sb, kT_sb)):
                    for qc in range(QC):
                        ps = bigps.tile([128, 1024], f32, tag="big")
                        for k in range(6):
                            nc.tensor.matmul(
                                ps[:, :512],
                                w_sb[:, k, hp * 128:(hp + 1) * 128],
                                xT_sb[:, k, qc * 512:(qc + 1) * 512],
                                start=(k == 0), stop=(k == 5))
                        nc.vector.tensor_scalar_add(
                            out=dst[:, hp, qc * 512:(qc + 1) * 512],
                            in0=ps[:, :512],
                            scalar1=b_sb[:, hp:hp + 1])

                # ---- attention for the two heads of this pair ----
                cA = 2 * hp * (HD + 1)
                cB = (2 * hp + 1) * (HD + 1)
                for qc in range(QC):
                    qs = slice(qc * 512, (qc + 1) * 512)
                    ovA = ovps.tile([65, 512], f32, tag="ovA")
                    ovB = ovps.tile([65, 512], f32, tag="ovB")
                    for kv in range(KT):
                        ks = slice(kv * 128, (kv + 1) * 128)
                        st = bigps.tile([128, 1024], f32, tag="big")
                        nc.tensor.matmul(st[:, 0:512],
                                         kT_sb[0:64, hp, ks],
                                         qT_sb[0:64, hp, qs],
                                         start=True, stop=True)
                        nc.tensor.matmul(st[:, 512:1024],
                                         kT_sb[64:128, hp, ks],
                                         qT_sb[64:128, hp, qs],
                                         start=True, stop=True)
                        pt = ptp.tile([128, 1024], f32r)
                        nc.scalar.activation(out=pt[:], in_=st[:], func=Exp,
                                             scale=0.125)
                        nc.tensor.matmul(ovA[:], v_sb[:, kv, cA:cA + 65],
                                         pt[:, 0:512],
                                         start=(kv == 0), stop=(kv == KT - 1),
                                         skip_group_check=True)
                        nc.tensor.matmul(ovB[:], v_sb[:, kv, cB:cB + 65],
                                         pt[:, 512:1024],
                                         start=(kv == 0), stop=(kv == KT - 1),
                                         skip_group_check=True)
                    for (ov, half) in ((ovA, 0), (ovB, 1)):
                        r = normp.tile([1, 512], f32, tag="r")
                        nc.vector.reciprocal(out=r[:], in_=ov[64:65, :])
                        rb = normp.tile([64, 512], f32, tag="rb")
                        nc.gpsimd.partition_broadcast(rb[:], r[:])
                        nc.vector.tensor_mul(
                            out=ohT_sb[half * 64:(half + 1) * 64, hp, qs],
                            in0=ov[0:64, :], in1=rb[:])

            # ---- partial output projection ----
            for m in range(KT):
                py = bigps.tile([128, 1024], f32, tag="big")
                for hs in (slice(0, 512), slice(512, 768)):
                    for t in range(3):
                        nc.tensor.matmul(py[:, hs],
                                         ohT_sb[:, t, m * 128:(m + 1) * 128],
                                         wp_sb[:, t, hs],
                                         start=(t == 0), stop=(t == 2))
                ysb = youtp.tile([128, D], f32)
                nc.vector.tensor_copy(out=ysb[:], in_=py[:, :D])
                nc.sync.dma_start(y.ap()[m * 128:(m + 1) * 128, :], ysb[:])

    nc.finalize()
    return nc


def _get_nc():
    if "nc" not in _compiled:
        _compiled["nc"] = _build_nc()
    return _compiled["nc"]


def _in_map_for_core(c, x, w_qkv, b_qkv, xT_cache):
    b, g = divmod(c, 2)
    if b not in xT_cache:
        xT_cache[b] = np.ascontiguousarray(x[b].T)
    wq = np.ascontiguousarray(w_qkv[:, g * 384:(g + 1) * 384])
    wk = np.ascontiguousarray(w_qkv[:, 768 + g * 384:768 + (g + 1) * 384])
    wv = np.zeros((D, VW), np.float32)
    bv = np.zeros((VW,), np.float32)
    for h in range(HPG):
        gh = g * HPG + h
        wv[:, h * 65:h * 65 + 64] = w_qkv[:, 1536 + gh * 64:1536 + (gh + 1) * 64]
        bv[h * 65:h * 65 + 64] = b_qkv[1536 + gh * 64:1536 + (gh + 1) * 64]
        bv[h * 65 + 64] = 1.0
    return {
        "xT": xT_cache[b],
        "wq": wq,
        "wk": wk,
        "wv": wv,
        "bq": np.ascontiguousarray(b_qkv[g * 384:(g + 1) * 384]),
        "bk": np.ascontiguousarray(b_qkv[768 + g * 384:768 + (g + 1) * 384]),
        "bv": bv,
    }


def kernel(x, w_qkv, b_qkv, w_proj, b_proj):
    global LAST_RESULT
    from concourse.bass_utils import run_bass_kernel_spmd

    x = np.asarray(x, np.float32)
    w_qkv = np.asarray(w_qkv, np.float32)
    b_qkv = np.asarray(b_qkv, np.float32)
    w_proj = np.asarray(w_proj, np.float32)
    b_proj = np.asarray(b_proj, np.float32)

    nc = _get_nc()
    xT_cache = {}
    in_maps = []
    for c in range(NCORES):
        m = _in_map_for_core(c, x, w_qkv, b_qkv, xT_cache)
        g = c % 2
        m["wp"] = np.ascontiguousarray(w_proj[g * 384:(g + 1) * 384, :])
        in_maps.append(m)

    LAST_RESULT = run_bass_kernel_spmd(nc, in_maps,
                                       core_ids=list(range(NCORES)))
    out = np.empty((B, S, D), np.float32)
    for b in range(B):
        out[b] = (LAST_RESULT.results[2 * b]["y"]
                  + LAST_RESULT.results[2 * b + 1]["y"] + b_proj)
    return out


# revision 7
# speedup vs baseline: 1.2072x; 1.2072x over previous
"""Multi-head attention (B=4, S=2048, D=768, H=12) on 8 Trainium2 NeuronCores.

Sharding: batch x head-group. Core c handles batch b = c//2 and head group
g = c%2 (6 heads of 64 dims each). Each core computes its heads' QKV
projections, attention, and a partial output projection (contracting only its
384 head dims of w_proj). The host sums the two partial projections per batch
and adds b_proj.

Per-core device pipeline (all matmuls in fp32r = full-rate FP22):
  1. QKV: q^T,k^T stored [head_dim, S] (head dim on partitions), v stored
     [S, head_dim] with a ones column appended per head (for softmax sums).
  2. Attention per head pair (row-tiled K=64 matmuls share the PE array):
     S^T tile = k^T.T @ q^T -> PSUM, exp via ScalarE (scale=1/8 folded in),
     then out^T[65, q] += [v|1].T @ P^T accumulated over kv tiles; row 64 of
     out^T is the softmax denominator. Normalize with reciprocal +
     partition-broadcast + multiply into oh^T [head_dim, S].
  3. proj partial: y = oh^T.T @ w_proj_rows, DVE copy PSUM->SBUF, DMA out.
"""

import numpy as np

try:
    from ml_dtypes import bfloat16 as _bf16
except ImportError:
    _bf16 = None

B, S, D = 4, 2048, 768
H, HD = 12, 64
HPG = 6          # heads per group (per core)
NCORES = 8
KT = S // 128    # 16 kv tiles
QC = 4           # q chunks of 512
VW = HPG * (HD + 1)   # 390: v with ones column per head

_compiled = {}
LAST_RESULT = None


def _build_nc():
    import concourse.bass as bass
    import concourse.mybir as mybir
    import concourse.tile as tile
    from concourse import bacc

    f32 = mybir.dt.float32
    bf16 = mybir.dt.bfloat16
    Exp = mybir.ActivationFunctionType.Exp

    nc = bacc.Bacc("TRN2", target_bir_lowering=False, debug=False,
                   num_devices=NCORES)

    xT = nc.dram_tensor("xT", [D, S], bf16, kind="ExternalInput")
    wq = nc.dram_tensor("wq", [D, 384], bf16, kind="ExternalInput")
    wk = nc.dram_tensor("wk", [D, 384], bf16, kind="ExternalInput")
    wv = nc.dram_tensor("wv", [D, VW], bf16, kind="ExternalInput")
    bq = nc.dram_tensor("bq", [384], f32, kind="ExternalInput")
    bk = nc.dram_tensor("bk", [384], f32, kind="ExternalInput")
    bv = nc.dram_tensor("bv", [VW], bf16, kind="ExternalInput")
    wp = nc.dram_tensor("wp", [384, D], bf16, kind="ExternalInput")
    y = nc.dram_tensor("y", [S, D], f32, kind="ExternalOutput")

    with tile.TileContext(nc) as tc:
        with tc.tile_pool(name="singles", bufs=1) as singles, \
             tc.tile_pool(name="pt", bufs=2) as ptp, \
             tc.tile_pool(name="norm", bufs=2) as normp, \
             tc.tile_pool(name="yout", bufs=2) as youtp, \
             tc.tile_pool(name="big", bufs=2, space="PSUM") as bigps, \
             tc.tile_pool(name="ov", bufs=2, space="PSUM") as ovps:

            # ---- static SBUF tensors ----
            xT_sb = singles.tile([128, 6, S], bf16)
            wq_sb = singles.tile([128, 6, 384], bf16)
            wk_sb = singles.tile([128, 6, 384], bf16)
            wv_sb = singles.tile([128, 6, VW], bf16)
            wp_sb = singles.tile([128, 3, D], bf16)
            bq_sb = singles.tile([128, 3], f32)
            bk_sb = singles.tile([128, 3], f32)
            bv_sb = singles.tile([1, VW], bf16)
            ones_col = singles.tile([1, 128], bf16)
            qT_sb = singles.tile([128, 3, S], bf16)
            kT_sb = singles.tile([128, 3, S], bf16)
            v_sb = singles.tile([128, KT, VW], bf16)
            ohT_sb = singles.tile([128, 3, S], bf16)

            xT_r = xT.ap().rearrange("(t p) q -> p t q", p=128)
            wq_r = wq.ap().rearrange("(t p) m -> p t m", p=128)
            wk_r = wk.ap().rearrange("(t p) m -> p t m", p=128)
            wv_r = wv.ap().rearrange("(t p) m -> p t m", p=128)
            wp_r = wp.ap().rearrange("(t p) m -> p t m", p=128)
            for t in range(6):
                nc.sync.dma_start(xT_sb[:, t, :], xT_r[:, t, :])
                nc.sync.dma_start(wq_sb[:, t, :], wq_r[:, t, :])
                nc.sync.dma_start(wk_sb[:, t, :], wk_r[:, t, :])
                nc.sync.dma_start(wv_sb[:, t, :], wv_r[:, t, :])
            for t in range(3):
                nc.sync.dma_start(wp_sb[:, t, :], wp_r[:, t, :])
            nc.sync.dma_start(bq_sb[:], bq.ap().rearrange("(t p) -> p t", p=128))
            nc.sync.dma_start(bk_sb[:], bk.ap().rearrange("(t p) -> p t", p=128))
            nc.sync.dma_start(bv_sb[:], bv.ap()[None, :])
            ones_f32 = singles.tile([1, 128], f32)
            nc.vector.memset(ones_f32[:], 1.0)
            nc.vector.tensor_copy(out=ones_col[:], in_=ones_f32[:])

            # ---- v = x @ wv + bv (ones columns arrive via the bv row) ----
            for s in range(KT):
                vps = bigps.tile([128, 1024], f32, tag="big")
                nc.tensor.matmul(vps[:, :VW], ones_col[:], bv_sb[:],
                                 start=True, stop=False)
                for k in range(6):
                    nc.tensor.matmul(vps[:, :VW],
                                     xT_sb[:, k, s * 128:(s + 1) * 128],
                                     wv_sb[:, k, :],
                                     start=False, stop=(k == 5))
                nc.vector.tensor_copy(out=v_sb[:, s, :], in_=vps[:, :VW])

            for hp in range(3):
                # ---- q^T, k^T for this head pair ----
                for (w_sb, b_sb, dst) in ((wq_sb, bq_sb, qT_sb),
                                          (wk_sb, bk_sb, kT_sb)):
                    for qc in range(QC):
                        ps = bigps.tile([128, 1024], f32, tag="big")
                        for k in range(6):
                            nc.tensor.matmul(
                                ps[:, :512],
                                w_sb[:, k, hp * 128:(hp + 1) * 128],
                                xT_sb[:, k, qc * 512:(qc + 1) * 512],
                                start=(k == 0), stop=(k == 5))
                        nc.vector.tensor_scalar_add(
                            out=dst[:, hp, qc * 512:(qc + 1) * 512],
                            in0=ps[:, :512],
                            scalar1=b_sb[:, hp:hp + 1])

                # ---- attention for the two heads of this pair ----
                cA = 2 * hp * (HD + 1)
                cB = (2 * hp + 1) * (HD + 1)
                for qc in range(QC):
                    qs = slice(qc * 512, (qc + 1) * 512)
                    ovA = ovps.tile([65, 512], f32, tag="ovA")
                    ovB = ovps.tile([65, 512], f32, tag="ovB")
                    for kv in range(KT):
                        ks = slice(kv * 128, (kv + 1) * 128)
                        st = bigps.tile([128, 1024], f32, tag="big")
                        nc.tensor.matmul(st[:, 0:512],
                                         kT_sb[0:64, hp, ks],
                                         qT_sb[0:64, hp, qs],
                                         start=True, stop=True)
                        nc.tensor.matmul(st[:, 512:1024],
                                         kT_sb[64:128, hp, ks],
                                         qT_sb[64:128, hp, qs],
                                         start=True, stop=True)
                        pt = ptp.tile([128, 1024], bf16)
                        nc.scalar.activation(out=pt[:], in_=st[:], func=Exp,
                                             scale=0.125)
                        nc.tensor.matmul(ovA[:], v_sb[:, kv, cA:cA + 65],
                                         pt[:, 0:512],
                                         start=(kv == 0), stop=(kv == KT - 1),
                                         skip_group_check=True)
                        nc.tensor.matmul(ovB[:], v_sb[:, kv, cB:cB + 65],
                                         pt[:, 512:1024],
                                         start=(kv == 0), stop=(kv == KT - 1),
                                         skip_group_check=True)
                    den = normp.tile([1, 1024], f32, tag="den")
                    nc.vector.tensor_copy(out=den[:, 0:512], in_=ovA[64:65, :])
                    nc.vector.tensor_copy(out=den[:, 512:1024], in_=ovB[64:65, :])
                    rcp = normp.tile([1, 1024], f32, tag="rcp")
                    nc.vector.reciprocal_approx_fast(out=rcp[:], in_=den[:])
                    for (ov, half) in ((ovA, 0), (ovB, 1)):
                        rb = normp.tile([64, 512], f32, tag="rb")
                        nc.gpsimd.partition_broadcast(
                            rb[:], rcp[:, half * 512:(half + 1) * 512])
                        nc.vector.tensor_mul(
                            out=ohT_sb[half * 64:(half + 1) * 64, hp, qs],
                            in0=ov[0:64, :], in1=rb[:])

            # ---- partial output projection ----
            for m in range(KT):
                py = bigps.tile([128, 1024], f32, tag="big")
                for hs in (slice(0, 512), slice(512, 768)):
                    for t in range(3):
                        nc.tensor.matmul(py[:, hs],
                                         ohT_sb[:, t, m * 128:(m + 1) * 128],
                                         wp_sb[:, t, hs],
                                         start=(t == 0), stop=(t == 2))
                ysb = youtp.tile([128, D], f32)
                nc.vector.tensor_copy(out=ysb[:], in_=py[:, :D])
                nc.sync.dma_start(y.ap()[m * 128:(m + 1) * 128, :], ysb[:])

    nc.finalize()
    return nc


def _get_nc():
    if "nc" not in _compiled:
        _compiled["nc"] = _build_nc()
    return _compiled["nc"]


def _in_map_for_core(c, x, w_qkv, b_qkv, xT_cache):
    b, g = divmod(c, 2)
    if b not in xT_cache:
        xT_cache[b] = np.ascontiguousarray(x[b].T).astype(_bf16)
    wq = np.ascontiguousarray(w_qkv[:, g * 384:(g + 1) * 384])
    wk = np.ascontiguousarray(w_qkv[:, 768 + g * 384:768 + (g + 1) * 384])
    wv = np.zeros((D, VW), np.float32)
    bv = np.zeros((VW,), np.float32)
    for h in range(HPG):
        gh = g * HPG + h
        wv[:, h * 65:h * 65 + 64] = w_qkv[:, 1536 + gh * 64:1536 + (gh + 1) * 64]
        bv[h * 65:h * 65 + 64] = b_qkv[1536 + gh * 64:1536 + (gh + 1) * 64]
        bv[h * 65 + 64] = 1.0
    return {
        "xT": xT_cache[b],
        "wq": wq.astype(_bf16),
        "wk": wk.astype(_bf16),
        "wv": wv.astype(_bf16),
        "bq": np.ascontiguousarray(b_qkv[g * 384:(g + 1) * 384]),
        "bk": np.ascontiguousarray(b_qkv[768 + g * 384:768 + (g + 1) * 384]),
        "bv": bv.astype(_bf16),
    }


def kernel(x, w_qkv, b_qkv, w_proj, b_proj):
    global LAST_RESULT
    from concourse.bass_utils import run_bass_kernel_spmd

    x = np.asarray(x, np.float32)
    w_qkv = np.asarray(w_qkv, np.float32)
    b_qkv = np.asarray(b_qkv, np.float32)
    w_proj = np.asarray(w_proj, np.float32)
    b_proj = np.asarray(b_proj, np.float32)

    nc = _get_nc()
    xT_cache = {}
    in_maps = []
    for c in range(NCORES):
        m = _in_map_for_core(c, x, w_qkv, b_qkv, xT_cache)
        g = c % 2
        m["wp"] = np.ascontiguousarray(w_proj[g * 384:(g + 1) * 384, :]).astype(_bf16)
        in_maps.append(m)

    LAST_RESULT = run_bass_kernel_spmd(nc, in_maps,
                                       core_ids=list(range(NCORES)))
    out = np.empty((B, S, D), np.float32)
    for b in range(B):
        out[b] = (LAST_RESULT.results[2 * b]["y"]
                  + LAST_RESULT.results[2 * b + 1]["y"] + b_proj)
    return out


# revision 10
# speedup vs baseline: 1.2499x; 1.0354x over previous
"""Multi-head attention (B=4, S=2048, D=768, H=12) on 8 Trainium2 NeuronCores.

Sharding: batch x head-group. Core c handles batch b = c//2 and head group
g = c%2 (6 heads of 64 dims each). Each core computes its heads' QKV
projections, attention, and a partial output projection (contracting only its
384 head dims of w_proj). The host sums the two partial projections per batch
and adds b_proj.

Per-core device pipeline (matmul operands in bf16, accumulation in fp32):
  1. QKV: q^T,k^T stored [head_dim, S] (head dim on partitions), v stored
     [S, head_dim] with a ones column appended per head (for softmax sums).
  2. Attention per head pair (row-tiled K=64 matmuls share the PE array):
     S^T tile = k^T.T @ q^T -> PSUM, exp via ScalarE (scale=1/8 folded in),
     then out^T[65, q] += [v|1].T @ P^T accumulated over kv tiles; row 64 of
     out^T is the softmax denominator. Normalize with reciprocal +
     partition-broadcast + multiply into oh^T [head_dim, S].
  3. proj partial: y = oh^T.T @ w_proj_rows, DVE copy PSUM->SBUF, DMA out.
     Emitted interleaved with the last pair's attention so it fills PE slack.
"""

import numpy as np

from ml_dtypes import bfloat16 as _bf16

B, S, D = 4, 2048, 768
H, HD = 12, 64
HPG = 6          # heads per group (per core)
NCORES = 8
KT = S // 128    # 16 kv tiles
QC = 4           # q chunks of 512
VW = HPG * (HD + 1)   # 390: v with ones column per head

_compiled = {}
LAST_RESULT = None


def _build_nc():
    import concourse.mybir as mybir
    import concourse.tile as tile
    from concourse import bacc

    f32 = mybir.dt.float32
    bf16 = mybir.dt.bfloat16
    Exp = mybir.ActivationFunctionType.Exp

    nc = bacc.Bacc("TRN2", target_bir_lowering=False, debug=False,
                   num_devices=NCORES)

    xT = nc.dram_tensor("xT", [D, S], bf16, kind="ExternalInput")
    wq = nc.dram_tensor("wq", [D, 384], bf16, kind="ExternalInput")
    wk = nc.dram_tensor("wk", [D, 384], bf16, kind="ExternalInput")
    wv = nc.dram_tensor("wv", [D, VW], bf16, kind="ExternalInput")
    bq = nc.dram_tensor("bq", [384], f32, kind="ExternalInput")
    bk = nc.dram_tensor("bk", [384], f32, kind="ExternalInput")
    bv = nc.dram_tensor("bv", [VW], bf16, kind="ExternalInput")
    wp = nc.dram_tensor("wp", [384, D], bf16, kind="ExternalInput")
    y = nc.dram_tensor("y", [S, D], f32, kind="ExternalOutput")

    with tile.TileContext(nc) as tc:
        with tc.tile_pool(name="singles", bufs=1) as singles, \
             tc.tile_pool(name="pt", bufs=4) as ptp, \
             tc.tile_pool(name="norm", bufs=2) as normp, \
             tc.tile_pool(name="yout", bufs=2) as youtp, \
             tc.tile_pool(name="stp", bufs=2, space="PSUM") as stp, \
             tc.tile_pool(name="qkvp", bufs=2, space="PSUM") as qkvp, \
             tc.tile_pool(name="ovp", bufs=1, space="PSUM") as ovp:

            # ---- static SBUF tensors ----
            xT_sb = singles.tile([128, 6, S], bf16)
            wq_sb = singles.tile([128, 6, 384], bf16)
            wk_sb = singles.tile([128, 6, 384], bf16)
            wv_sb = singles.tile([128, 6, VW], bf16)
            wp_sb = singles.tile([128, 3, D], bf16)
            bq_sb = singles.tile([128, 3], f32)
            bk_sb = singles.tile([128, 3], f32)
            bv_sb = singles.tile([1, VW], bf16)
            ones_col = singles.tile([1, 128], bf16)
            qT_sb = singles.tile([128, 3, S], bf16)
            kT_sb = singles.tile([128, 3, S], bf16)
            v_sb = singles.tile([128, KT, VW], bf16)
            ohT_sb = singles.tile([128, 3, S], bf16)

            # input DMAs: single big transfers; critical ones first
            nc.sync.dma_start(xT_sb[:],
                              xT.ap().rearrange("(t p) q -> p t q", p=128))
            nc.sync.dma_start(wq_sb[:],
                              wq.ap().rearrange("(t p) m -> p t m", p=128))
            nc.sync.dma_start(wk_sb[:],
                              wk.ap().rearrange("(t p) m -> p t m", p=128))
            nc.gpsimd.dma_start(bq_sb[:],
                                bq.ap().rearrange("(t p) -> p t", p=128))
            nc.gpsimd.dma_start(bk_sb[:],
                                bk.ap().rearrange("(t p) -> p t", p=128))
            nc.gpsimd.dma_start(bv_sb[:], bv.ap()[None, :])
            nc.sync.dma_start(wv_sb[:],
                              wv.ap().rearrange("(t p) m -> p t m", p=128))
            nc.sync.dma_start(wp_sb[:],
                              wp.ap().rearrange("(t p) m -> p t m", p=128))
            ones_f32 = singles.tile([1, 128], f32)
            nc.vector.memset(ones_f32[:], 1.0)
            nc.vector.tensor_copy(out=ones_col[:], in_=ones_f32[:])

            # ---- helper emitters (Tile executes ~ emission order) ----
            def emit_qkT(hp, which, qc):
                (w_sb, b_sb, dst) = ((wq_sb, bq_sb, qT_sb),
                                     (wk_sb, bk_sb, kT_sb))[which]
                ps = qkvp.tile([128, 512], f32, tag="qkv")
                for k in range(6):
                    nc.tensor.matmul(
                        ps[:],
                        w_sb[:, k, hp * 128:(hp + 1) * 128],
                        xT_sb[:, k, qc * 512:(qc + 1) * 512],
                        start=(k == 0), stop=(k == 5))
                nc.vector.tensor_scalar_add(
                    out=dst[:, hp, qc * 512:(qc + 1) * 512],
                    in0=ps[:],
                    scalar1=b_sb[:, hp:hp + 1])

            def emit_v(kv):
                vps = qkvp.tile([128, 512], f32, tag="qkv")
                nc.tensor.matmul(vps[:, :VW], ones_col[:], bv_sb[:],
                                 start=True, stop=False)
                for k in range(6):
                    nc.tensor.matmul(vps[:, :VW],
                                     xT_sb[:, k, kv * 128:(kv + 1) * 128],
                                     wv_sb[:, k, :],
                                     start=False, stop=(k == 5))
                nc.vector.tensor_copy(out=v_sb[:, kv, :], in_=vps[:, :VW])

            def emit_proj(m):
                ysb = youtp.tile([128, D], f32)
                for (hs, n) in ((slice(0, 512), 512), (slice(512, 768), 256)):
                    py = qkvp.tile([128, 512], f32, tag="qkv")
                    for t in range(3):
                        nc.tensor.matmul(py[:, :n],
                                         ohT_sb[:, t, m * 128:(m + 1) * 128],
                                         wp_sb[:, t, hs],
                                         start=(t == 0), stop=(t == 2))
                    nc.vector.tensor_copy(out=ysb[:, hs], in_=py[:, :n])
                nc.sync.dma_start(y.ap()[m * 128:(m + 1) * 128, :], ysb[:])

            # q^T/k^T for pair 0 must precede its attention
            for qc in range(QC):
                emit_qkT(0, 0, qc)
                emit_qkT(0, 1, qc)

            # ---- attention per head pair, with filler work interleaved ----
            # filler[hp] yields emitters sprinkled into pair hp's attention
            filler = {
                0: [(lambda kv=kv: emit_v(kv)) for kv in range(KT)]
                   + [(lambda w=w, qc=qc: emit_qkT(1, w, qc))
                      for qc in range(QC) for w in range(2)],
                1: [(lambda w=w, qc=qc: emit_qkT(2, w, qc))
                    for qc in range(QC) for w in range(2)],
                2: [],
            }
            for hp in range(3):
                cA = 2 * hp * (HD + 1)
                cB = (2 * hp + 1) * (HD + 1)
                fill = list(filler[hp])
                for qc in range(QC):
                    qs = slice(qc * 512, (qc + 1) * 512)
                    ov = ovp.tile([65, 1024], f32, tag="ov")
                    for kv in range(KT):
                        if hp == 0 and qc == 0:
                            # v tile kv is consumed by this iteration's PV
                            fill.pop(0)()
                        elif fill and kv % 4 == 3:
                            fill.pop(0)()
                        if hp == 2 and qc > 0 and kv % 4 == 3:
                            emit_proj((qc - 1) * 4 + kv // 4)
                        ks = slice(kv * 128, (kv + 1) * 128)
                        st = stp.tile([128, 1024], f32, tag="st")
                        nc.tensor.matmul(st[:, 0:512],
                                         kT_sb[0:64, hp, ks],
                                         qT_sb[0:64, hp, qs],
                                         start=True, stop=True)
                        nc.tensor.matmul(st[:, 512:1024],
                                         kT_sb[64:128, hp, ks],
                                         qT_sb[64:128, hp, qs],
                                         start=True, stop=True)
                        pt = ptp.tile([128, 1024], bf16)
                        nc.scalar.activation(out=pt[:], in_=st[:], func=Exp,
                                             scale=0.125)
                        nc.tensor.matmul(ov[:, 0:512],
                                         v_sb[:, kv, cA:cA + 65],
                                         pt[:, 0:512],
                                         start=(kv == 0), stop=(kv == KT - 1),
                                         skip_group_check=True)
                        nc.tensor.matmul(ov[:, 512:1024],
                                         v_sb[:, kv, cB:cB + 65],
                                         pt[:, 512:1024],
                                         start=(kv == 0), stop=(kv == KT - 1),
                                         skip_group_check=True)
                    den = normp.tile([1, 1024], f32, tag="den")
                    nc.vector.tensor_copy(out=den[:], in_=ov[64:65, :])
                    rcp = normp.tile([1, 1024], f32, tag="rcp")
                    nc.vector.reciprocal_approx_fast(out=rcp[:], in_=den[:])
                    for half in range(2):
                        rb = normp.tile([64, 512], f32, tag="rb")
                        nc.gpsimd.partition_broadcast(
                            rb[:], rcp[:, half * 512:(half + 1) * 512])
                        nc.vector.tensor_mul(
                            out=ohT_sb[half * 64:(half + 1) * 64, hp, qs],
                            in0=ov[0:64, half * 512:(half + 1) * 512],
                            in1=rb[:])
            # tail: last q-chunk's projection
            for m in range(12, 16):
                emit_proj(m)

    nc.finalize()
    return nc


def _get_nc():
    if "nc" not in _compiled:
        _compiled["nc"] = _build_nc()
    return _compiled["nc"]


def _in_map_for_core(c, x, w_qkv, b_qkv, xT_cache):
    b, g = divmod(c, 2)
    if b not in xT_cache:
        xT_cache[b] = np.ascontiguousarray(x[b].T).astype(_bf16)
    wq = np.ascontiguousarray(w_qkv[:, g * 384:(g + 1) * 384])
    wk = np.ascontiguousarray(w_qkv[:, 768 + g * 384:768 + (g + 1) * 384])
    wv = np.zeros((D, VW), np.float32)
    bv = np.zeros((VW,), np.float32)
    for h in range(HPG):
        gh = g * HPG + h
        wv[:, h * 65:h * 65 + 64] = w_qkv[:, 1536 + gh * 64:1536 + (gh + 1) * 64]
        bv[h * 65:h * 65 + 64] = b_qkv[1536 + gh * 64:1536 + (gh + 1) * 64]
        bv[h * 65 + 64] = 1.0
    return {
        "xT": xT_cache[b],
        "wq": wq.astype(_bf16),
        "wk": wk.astype(_bf16),
        "wv": wv.astype(_bf16),
        "bq": np.ascontiguousarray(b_qkv[g * 384:(g + 1) * 384]),
        "bk": np.ascontiguousarray(b_qkv[768 + g * 384:768 + (g + 1) * 384]),
        "bv": bv.astype(_bf16),
    }


def kernel(x, w_qkv, b_qkv, w_proj, b_proj):
    global LAST_RESULT
    from concourse.bass_utils import run_bass_kernel_spmd

    x = np.asarray(x, np.float32)
    w_qkv = np.asarray(w_qkv, np.float32)
    b_qkv = np.asarray(b_qkv, np.float32)
    w_proj = np.asarray(w_proj, np.float32)
    b_proj = np.asarray(b_proj, np.float32)

    nc = _get_nc()
    xT_cache = {}
    in_maps = []
    for c in range(NCORES):
        m = _in_map_for_core(c, x, w_qkv, b_qkv, xT_cache)
        g = c % 2
        m["wp"] = np.ascontiguousarray(
            w_proj[g * 384:(g + 1) * 384, :]).astype(_bf16)
        in_maps.append(m)

    LAST_RESULT = run_bass_kernel_spmd(nc, in_maps,
                                       core_ids=list(range(NCORES)))
    out = np.empty((B, S, D), np.float32)
    for b in range(B):
        out[b] = (LAST_RESULT.results[2 * b]["y"]
                  + LAST_RESULT.results[2 * b + 1]["y"] + b_proj)
    return out


# revision 15
# speedup vs baseline: 1.2735x; 1.0189x over previous
"""Multi-head attention (B=4, S=2048, D=768, H=12) on 8 Trainium2 NeuronCores.

Sharding: batch x head-group. Core c handles batch b = c//2 and head group
g = c%2 (6 heads of 64 dims each). Each core computes its heads' QKV
projections, attention, and a partial output projection (contracting only its
384 head dims of w_proj). The host sums the two partial projections per batch
and adds b_proj.

Per-core device pipeline (matmul operands in bf16, accumulation in fp32):
  1. QKV: q^T,k^T stored [head_dim, S] (head dim on partitions), v stored
     [S, head_dim] with a ones column appended per head (for softmax sums).
  2. Attention per head pair (row-tiled K=64 matmuls share the PE array):
     S^T tile = k^T.T @ q^T -> PSUM, exp via ScalarE (scale=1/8 folded in),
     then out^T[65, q] += [v|1].T @ P^T accumulated over kv tiles; row 64 of
     out^T is the softmax denominator. Normalize with reciprocal +
     partition-broadcast + multiply into oh^T [head_dim, S].
  3. proj partial: y = oh^T.T @ w_proj_rows, DVE copy PSUM->SBUF, DMA out.
     Emitted interleaved with the last pair's attention so it fills PE slack.
"""

import numpy as np

from ml_dtypes import bfloat16 as _bf16

B, S, D = 4, 2048, 768
H, HD = 12, 64
HPG = 6          # heads per group (per core)
NCORES = 8
KT = S // 128    # 16 kv tiles
QC = 4           # q chunks of 512
VW = HPG * (HD + 1)   # 390: v with ones column per head

_compiled = {}
LAST_RESULT = None


def _build_nc():
    import concourse.mybir as mybir
    import concourse.tile as tile
    from concourse import bacc

    f32 = mybir.dt.float32
    bf16 = mybir.dt.bfloat16
    Exp = mybir.ActivationFunctionType.Exp

    nc = bacc.Bacc("TRN2", target_bir_lowering=False, debug=False,
                   num_devices=NCORES)

    xT = nc.dram_tensor("xT", [D, S], bf16, kind="ExternalInput")
    wq = nc.dram_tensor("wq", [D, 384], bf16, kind="ExternalInput")
    wk = nc.dram_tensor("wk", [D, 384], bf16, kind="ExternalInput")
    wv = nc.dram_tensor("wv", [D, VW], bf16, kind="ExternalInput")
    bq = nc.dram_tensor("bq", [384], f32, kind="ExternalInput")
    bk = nc.dram_tensor("bk", [384], f32, kind="ExternalInput")
    bv = nc.dram_tensor("bv", [VW], bf16, kind="ExternalInput")
    wp = nc.dram_tensor("wp", [384, D], bf16, kind="ExternalInput")
    y = nc.dram_tensor("y", [S, D], f32, kind="ExternalOutput")

    with tile.TileContext(nc) as tc:
        with tc.tile_pool(name="singles", bufs=1) as singles, \
             tc.tile_pool(name="pt", bufs=6) as ptp, \
             tc.tile_pool(name="norm", bufs=3) as normp, \
             tc.tile_pool(name="yout", bufs=2) as youtp, \
             tc.tile_pool(name="stp", bufs=2, space="PSUM") as stp, \
             tc.tile_pool(name="qkvp", bufs=2, space="PSUM") as qkvp, \
             tc.tile_pool(name="ovp", bufs=1, space="PSUM") as ovp:

            # ---- static SBUF tensors ----
            xT_sb = singles.tile([128, 6, S], bf16)
            wq_sb = singles.tile([128, 6, 384], bf16)
            wk_sb = singles.tile([128, 6, 384], bf16)
            wv_sb = singles.tile([128, 6, VW], bf16)
            wp_sb = singles.tile([128, 3, D], bf16)
            bq_sb = singles.tile([128, 3], f32)
            bk_sb = singles.tile([128, 3], f32)
            bv_sb = singles.tile([1, VW], bf16)
            ones_col = singles.tile([1, 128], bf16)
            qT_sb = singles.tile([128, 3, S], bf16)
            kT_sb = singles.tile([128, 3, S], bf16)
            v_sb = singles.tile([128, KT, VW], bf16)
            ohT_sb = singles.tile([128, 3, S], bf16)

            # input DMAs: weights first, xT split per k-chunk so the
            # first projection groups can start while later chunks stream
            nc.sync.dma_start(wk_sb[:],
                              wk.ap().rearrange("(t p) m -> p t m", p=128))
            nc.sync.dma_start(wq_sb[:],
                              wq.ap().rearrange("(t p) m -> p t m", p=128))
            xT_r = xT.ap().rearrange("(t p) q -> p t q", p=128)
            for t in range(6):
                nc.sync.dma_start(xT_sb[:, t, :], xT_r[:, t, :])
            nc.gpsimd.dma_start(bq_sb[:],
                                bq.ap().rearrange("(t p) -> p t", p=128))
            nc.gpsimd.dma_start(bk_sb[:],
                                bk.ap().rearrange("(t p) -> p t", p=128))
            nc.gpsimd.dma_start(bv_sb[:], bv.ap()[None, :])
            nc.sync.dma_start(wv_sb[:],
                              wv.ap().rearrange("(t p) m -> p t m", p=128))
            nc.sync.dma_start(wp_sb[:],
                              wp.ap().rearrange("(t p) m -> p t m", p=128))
            ones_f32 = singles.tile([1, 128], f32)
            nc.vector.memset(ones_f32[:], 1.0)
            nc.vector.tensor_copy(out=ones_col[:], in_=ones_f32[:])

            # ---- helper emitters (Tile executes ~ emission order) ----
            def emit_qkT(hp, which, qc):
                (w_sb, b_sb, dst) = ((wq_sb, bq_sb, qT_sb),
                                     (wk_sb, bk_sb, kT_sb))[which]
                ps = qkvp.tile([128, 512], f32, tag="qkv")
                for k in range(6):
                    nc.tensor.matmul(
                        ps[:],
                        w_sb[:, k, hp * 128:(hp + 1) * 128],
                        xT_sb[:, k, qc * 512:(qc + 1) * 512],
                        start=(k == 0), stop=(k == 5))
                nc.vector.tensor_scalar_add(
                    out=dst[:, hp, qc * 512:(qc + 1) * 512],
                    in0=ps[:],
                    scalar1=b_sb[:, hp:hp + 1])

            def emit_v(kv):
                vps = qkvp.tile([128, 512], f32, tag="qkv")
                nc.tensor.matmul(vps[:, :VW], ones_col[:], bv_sb[:],
                                 start=True, stop=False)
                for k in range(6):
                    nc.tensor.matmul(vps[:, :VW],
                                     xT_sb[:, k, kv * 128:(kv + 1) * 128],
                                     wv_sb[:, k, :],
                                     start=False, stop=(k == 5))
                nc.vector.tensor_copy(out=v_sb[:, kv, :], in_=vps[:, :VW])

            def emit_proj(m):
                py = stp.tile([128, 1024], f32, tag="st")
                for hs in (slice(0, 512), slice(512, 768)):
                    for t in range(3):
                        nc.tensor.matmul(py[:, hs],
                                         ohT_sb[:, t, m * 128:(m + 1) * 128],
                                         wp_sb[:, t, hs],
                                         start=(t == 0), stop=(t == 2))
                ysb = youtp.tile([128, D], f32)
                nc.vector.tensor_copy(out=ysb[:], in_=py[:, :D])
                nc.sync.dma_start(y.ap()[m * 128:(m + 1) * 128, :], ysb[:])

            # k^T fully, then q^T qc0 — the first S^T matmul needs all of
            # k^T for pair 0 but only the first q chunk
            for qc in range(QC):
                emit_qkT(0, 1, qc)
            for qc in range(QC):
                emit_qkT(0, 0, qc)

            # ---- deferred-work queue, drained into attention PE/DVE slack ----
            fill = []

            def emit_phase2(hp, qc, ovc):
                qs = slice(qc * 512, (qc + 1) * 512)

                def go():
                    den = normp.tile([1, 1024], f32, tag="den")
                    nc.vector.tensor_copy(out=den[:], in_=ovc[64:65, :])
                    rcp = normp.tile([1, 1024], f32, tag="rcp")
                    nc.vector.reciprocal_approx_fast(out=rcp[:], in_=den[:])
                    rb = normp.tile([64, 1024], f32, tag="rb")
                    nc.gpsimd.partition_broadcast(rb[:], rcp[:])
                    for half in range(2):
                        nc.vector.tensor_mul(
                            out=ohT_sb[half * 64:(half + 1) * 64, hp, qs],
                            in0=ovc[0:64, half * 512:(half + 1) * 512],
                            in1=rb[:, half * 512:(half + 1) * 512])
                return go

            for w in range(2):
                for qc in range(QC):
                    fill.append(lambda w=w, qc=qc: emit_qkT(1, w, qc))

            # ---- attention per head pair ----
            for hp in range(3):
                cA = 2 * hp * (HD + 1)
                cB = (2 * hp + 1) * (HD + 1)
                for qc in range(QC):
                    qs = slice(qc * 512, (qc + 1) * 512)
                    ov = ovp.tile([65, 1024], f32, tag="ov")
                    for kv in range(KT):
                        if hp == 0 and qc == 0:
                            # v tile kv is consumed by this iteration's PV
                            emit_v(kv)
                        elif fill and kv % 2 == 1:
                            fill.pop(0)()
                        ks = slice(kv * 128, (kv + 1) * 128)
                        st = stp.tile([128, 1024], f32, tag="st")
                        nc.tensor.matmul(st[:, 0:512],
                                         kT_sb[0:64, hp, ks],
                                         qT_sb[0:64, hp, qs],
                                         start=True, stop=True)
                        nc.tensor.matmul(st[:, 512:1024],
                                         kT_sb[64:128, hp, ks],
                                         qT_sb[64:128, hp, qs],
                                         start=True, stop=True)
                        pt = ptp.tile([128, 1024], bf16)
                        nc.scalar.activation(out=pt[:], in_=st[:], func=Exp,
                                             scale=0.125)
                        nc.tensor.matmul(ov[:, 0:512],
                                         v_sb[:, kv, cA:cA + 65],
                                         pt[:, 0:512],
                                         start=(kv == 0), stop=(kv == KT - 1),
                                         skip_group_check=True)
                        nc.tensor.matmul(ov[:, 512:1024],
                                         v_sb[:, kv, cB:cB + 65],
                                         pt[:, 512:1024],
                                         start=(kv == 0), stop=(kv == KT - 1),
                                         skip_group_check=True)
                    # release ov quickly; defer the normalize to the queue
                    ovc = normp.tile([65, 1024], f32, tag="ovc")
                    nc.vector.tensor_copy(out=ovc[:], in_=ov[:])
                    fill.append(emit_phase2(hp, qc, ovc))
                    if hp == 1 and qc < 2:
                        # queue pair-2 q^T/k^T early enough
                        for w in range(2):
                            for half in range(2):
                                fill.append(lambda w=w, c=2 * qc + half:
                                            emit_qkT(2, w, c))
                    if hp == 2:
                        for m in range(4 * qc, 4 * qc + 4):
                            fill.append(lambda m=m: emit_proj(m))

            while fill:
                fill.pop(0)()

    nc.finalize()
    return nc


def _get_nc():
    if "nc" not in _compiled:
        _compiled["nc"] = _build_nc()
    return _compiled["nc"]


def _in_map_for_core(c, x, w_qkv, b_qkv, xT_cache):
    b, g = divmod(c, 2)
    if b not in xT_cache:
        xT_cache[b] = np.ascontiguousarray(x[b].T).astype(_bf16)
    wq = np.ascontiguousarray(w_qkv[:, g * 384:(g + 1) * 384])
    wk = np.ascontiguousarray(w_qkv[:, 768 + g * 384:768 + (g + 1) * 384])
    wv = np.zeros((D, VW), np.float32)
    bv = np.zeros((VW,), np.float32)
    for h in range(HPG):
        gh = g * HPG + h
        wv[:, h * 65:h * 65 + 64] = w_qkv[:, 1536 + gh * 64:1536 + (gh + 1) * 64]
        bv[h * 65:h * 65 + 64] = b_qkv[1536 + gh * 64:1536 + (gh + 1) * 64]
        bv[h * 65 + 64] = 1.0
    return {
        "xT": xT_cache[b],
        "wq": wq.astype(_bf16),
        "wk": wk.astype(_bf16),
        "wv": wv.astype(_bf16),
        "bq": np.ascontiguousarray(b_qkv[g * 384:(g + 1) * 384]),
        "bk": np.ascontiguousarray(b_qkv[768 + g * 384:768 + (g + 1) * 384]),
        "bv": bv.astype(_bf16),
    }


def kernel(x, w_qkv, b_qkv, w_proj, b_proj):
    global LAST_RESULT
    from concourse.bass_utils import run_bass_kernel_spmd

    x = np.asarray(x, np.float32)
    w_qkv = np.asarray(w_qkv, np.float32)
    b_qkv = np.asarray(b_qkv, np.float32)
    w_proj = np.asarray(w_proj, np.float32)
    b_proj = np.asarray(b_proj, np.float32)

    nc = _get_nc()
    xT_cache = {}
    in_maps = []
    for c in range(NCORES):
        m = _in_map_for_core(c, x, w_qkv, b_qkv, xT_cache)
        g = c % 2
        m["wp"] = np.ascontiguousarray(
            w_proj[g * 384:(g + 1) * 384, :]).astype(_bf16)
        in_maps.append(m)

    LAST_RESULT = run_bass_kernel_spmd(nc, in_maps,
                                       core_ids=list(range(NCORES)))
    out = np.empty((B, S, D), np.float32)
    for b in range(B):
        out[b] = (LAST_RESULT.results[2 * b]["y"]
                  + LAST_RESULT.results[2 * b + 1]["y"] + b_proj)
    return out
